# revision 39
# baseline (speedup 1.0000x reference)
"""MoE kernel for 8 TRN2 NeuronCores.

Strategy (expert-parallel, routing-as-sharding):
  - Router (Linear-GELU-Linear-softmax-top2) runs on host in f64 numpy;
    verified to reproduce the jax f32 reference top-2 sets exactly.
  - Token tiles (128 tokens, single expert each) are bin-packed onto the
    8 cores in up to two uniform "segments" per core: segment A runs sA
    tiles with one expert's weights, segment B runs sB tiles with a second
    expert's weights (loaded mid-kernel, overlapped with compute).
  - Per-core Bass kernel: 3-layer expert MLP with LayerNorm+exact-GELU
    between layers, bf16 matmuls with f32 PSUM accumulation, LN stats read
    PSUM directly, combine-weight scaling fused into output eviction.
    Software-pipelined across tiles (3-stage skew) to keep the PE busy.
  - Host scatter-adds the two expert contributions per token.
"""

import math
import os

import numpy as np

D, H, E, K = 512, 2048, 8, 2
EPS = 1e-5
P = 128

last_exec_time_ns = None


def _gelu_exact(x):
    from scipy.special import erf

    return 0.5 * x * (1.0 + erf(x / np.sqrt(2.0)))


def _route(t, Wg1, bg1, Wg2, bg2):
    th = t.astype(np.float64)
    h = th @ Wg1.astype(np.float64) + bg1.astype(np.float64)
    h = _gelu_exact(h)
    logits = h @ Wg2.astype(np.float64) + bg2.astype(np.float64)
    logits = logits - logits.max(axis=-1, keepdims=True)
    ex = np.exp(logits)
    gates = ex / ex.sum(axis=-1, keepdims=True)
    top2 = np.argsort(-gates, axis=-1, kind="stable")[:, :K]
    topv = np.take_along_axis(gates, top2, axis=-1)
    topv = topv / topv.sum(axis=-1, keepdims=True)
    return top2, topv.astype(np.float32)


def _pack_segments(tiles, n_slots=8):
    """Find minimal S and split S = sA + sB such that every expert's tile
    count can be covered by a_e A-slots (sA tiles each) + b_e B-slots (sB
    tiles each) with sum(a) <= n_slots, sum(b) <= n_slots.

    Returns (sA, sB, assign) where assign[e] = (a_e, b_e)."""
    total = sum(tiles)
    s_lo = max(1, (total + n_slots - 1) // n_slots)
    for S in range(s_lo, max(tiles) + 1):
        for sA in range(S, (S - 1) // 2, -1):
            sB = S - sA
            states = {(0, 0): 0}
            back = []
            ok = True
            for t in tiles:
                opts = []
                for a in range(n_slots + 1):
                    for b in range(n_slots + 1):
                        cap = a * sA + b * sB
                        if cap >= t:
                            opts.append((a, b, cap - t))
                new = {}
                for (au, bu), w in states.items():
                    for a, b, waste in opts:
                        if au + a <= n_slots and bu + b <= n_slots:
                            key = (au + a, bu + b)
                            val = (w + waste, (au, bu), (a, b))
                            if key not in new or new[key][0] > val[0]:
                                new[key] = val
                if not new:
                    ok = False
                    break
                back.append(new)
                states = {k: v[0] for k, v in new.items()}
            if not ok:
                continue
            key = min(states, key=lambda k: states[k])
            assign = []
            for st in reversed(back):
                w, prev, ab = st[key]
                assign.append(ab)
                key = prev
            return sA, sB, list(reversed(assign))
    return max(tiles), 0, [(1, 0)] * len(tiles)


def _build_program(n_a, n_b, affine, compute_dt_name="bfloat16"):
    """Per-core Bass program: n_a tiles with weight-set A, then n_b tiles
    with weight-set B (B weights streamed in mid-kernel)."""
    from concourse import bacc, bass, tile, mybir
    from concourse import masks

    f32 = mybir.dt.float32
    bf16 = getattr(mybir.dt, compute_dt_name)
    AF = mybir.ActivationFunctionType
    ALU = mybir.AluOpType

    n_tiles = n_a + n_b
    C = n_tiles * P
    two_seg = n_b > 0
    KD = D // P
    KH = H // P
    CS = 512

    nc = bacc.Bacc(None, target_bir_lowering=False, debug=False)

    tT_d = nc.dram_tensor("tT", (KD, P, C), bf16, kind="ExternalInput")
    w1a_d = nc.dram_tensor("W1a", (KD, P, H), bf16, kind="ExternalInput")
    w2a_d = nc.dram_tensor("W2a", (KH, P, H), bf16, kind="ExternalInput")
    w3a_d = nc.dram_tensor("W3a", (KH, P, D), bf16, kind="ExternalInput")
    cw_d = nc.dram_tensor("cw", (P, n_tiles), f32, kind="ExternalInput")
    out_d = nc.dram_tensor("out", (C, D), f32, kind="ExternalOutput")
    if two_seg:
        w1b_d = nc.dram_tensor("W1b", (KD, P, H), bf16, kind="ExternalInput")
        w2b_d = nc.dram_tensor("W2b", (KH, P, H), bf16, kind="ExternalInput")
        w3b_d = nc.dram_tensor("W3b", (KH, P, D), bf16, kind="ExternalInput")

    aff_d = {}
    for name, width in (
        ("b1", H), ("g1", H), ("be1", H),
        ("b2", H), ("g2", H), ("be2", H),
        ("b3", D),
    ):
        if affine[name]:
            aff_d[name] = nc.dram_tensor(name, (P, width), f32, kind="ExternalInput")

    with tile.TileContext(nc) as tc:
        with (
            tc.tile_pool(name="const", bufs=1) as const_pool,
            tc.tile_pool(name="hraw", bufs=2) as hraw_pool,
            tc.tile_pool(name="xg1", bufs=2) as xg1_pool,
            tc.tile_pool(name="xg2", bufs=2) as xg2_pool,
            tc.tile_pool(name="hT", bufs=2) as hT_pool,
            tc.tile_pool(name="outp", bufs=2) as out_pool,
            tc.tile_pool(name="st", bufs=3) as st_pool,
            tc.tile_pool(name="acc", bufs=6, space="PSUM") as acc_pool,
            tc.tile_pool(name="tp", bufs=2, space="PSUM") as tp_pool,
        ):
            # ---- resident loads (segment A + shared) ----
            w1a_s = const_pool.tile((P, KD, H), bf16)
            w2a_s = const_pool.tile((P, KH, H), bf16)
            w3a_s = const_pool.tile((P, KH, D), bf16)
            tT_s = const_pool.tile((P, KD, C), bf16)
            cw_s = const_pool.tile((P, n_tiles), f32)
            # spread resident loads over three queues so tile-0 compute and
            # the large W2 load proceed in parallel
            for k in range(KD):
                nc.scalar.dma_start(w1a_s[:, k, :], w1a_d[k])
            for k in range(KD):
                nc.sync.dma_start(tT_s[:, k, :], tT_d[k])
            for k in range(KH):
                nc.gpsimd.dma_start(w2a_s[:, k, :], w2a_d[k])
            for k in range(KH):
                nc.sync.dma_start(w3a_s[:, k, :], w3a_d[k])
            nc.sync.dma_start(cw_s[:], cw_d[:])
            if two_seg:
                w1b_s = const_pool.tile((P, KD, H), bf16)
                w3b_s = const_pool.tile((P, KH, D), bf16)
                for k in range(KD):
                    nc.sync.dma_start(w1b_s[:, k, :], w1b_d[k])
                for k in range(KH):
                    nc.sync.dma_start(w3b_s[:, k, :], w3b_d[k])

            identity = const_pool.tile((P, P), bf16)
            masks.make_identity(nc, identity[:])

            eps_t = const_pool.tile((P, 1), f32, name="eps_t")
            nc.vector.memset(eps_t[:], float(EPS))

            aff_s = {}
            for name in aff_d:
                width = aff_d[name].shape[1]
                row = const_pool.tile((P, width), f32, name=f"{name}_bcast")
                nc.sync.dma_start(row[:], aff_d[name][:])
                aff_s[name] = row

            def weights_for(i):
                if (not two_seg) or i < n_a:
                    return w1a_s, w2a_s, w3a_s
                return w1b_s, w2a_s, w3b_s

            def mm_ln_gelu(tile_i, lhsT_getter, n_k, w_s, nh, bname, gname, bename, xg_tag):
                """matmul (-> +b) -> LN -> (*g +be) -> gelu; returns xg tile."""
                nch = nh // CS
                fast = not (affine[bname] or affine[gname] or affine[bename])
                hraw = None
                if not fast:
                    hraw = hraw_pool.tile((P, nh), f32, tag="hraw")
                stats = st_pool.tile((P, nch, 6), f32, tag="stats")
                ps_list = []
                for c in range(nch):
                    ps = acc_pool.tile((P, CS), f32, name="ps_acc", tag="ps_acc")
                    for k in range(n_k):
                        nc.tensor.matmul(
                            ps[:],
                            lhsT_getter(k),
                            w_s[:, k, c * CS:(c + 1) * CS],
                            start=(k == 0),
                            stop=(k == n_k - 1),
                        )
                    cs_sl = slice(c * CS, (c + 1) * CS)
                    if fast:
                        nc.vector.bn_stats(stats[:, c, :], ps[:])
                        ps_list.append(ps)
                    else:
                        nc.scalar.copy(hraw[:, cs_sl], ps[:])
                        if affine[bname]:
                            nc.vector.tensor_tensor(
                                out=hraw[:, cs_sl], in0=hraw[:, cs_sl],
                                in1=aff_s[bname][:, cs_sl], op=ALU.add,
                            )
                        nc.vector.bn_stats(stats[:, c, :], hraw[:, cs_sl])
                mv = st_pool.tile((P, 2), f32, tag="mv")
                nc.vector.bn_aggr(mv[:], stats[:])
                sd = st_pool.tile((P, 1), f32, tag="sd")
                nc.scalar.activation(sd[:], mv[:, 1:2], AF.Sqrt, bias=eps_t[:])
                rstd = st_pool.tile((P, 1), f32, tag="rstd")
                nc.vector.reciprocal(rstd[:], sd[:])
                negmr = st_pool.tile((P, 1), f32, tag="negmr")
                nc.vector.tensor_scalar(
                    out=negmr[:], in0=mv[:, 0:1], scalar1=rstd[:], scalar2=-1.0,
                    op0=ALU.mult, op1=ALU.mult,
                )
                pool = xg1_pool if xg_tag == "xg1" else xg2_pool
                xg = pool.tile((P, nh), bf16, tag=xg_tag)
                for c in range(nch):
                    cs_sl = slice(c * CS, (c + 1) * CS)
                    if fast:
                        nc.scalar.activation(
                            xg[:, cs_sl], ps_list[c][:], AF.Gelu,
                            bias=negmr[:], scale=rstd[:],
                        )
                    else:
                        xn = hraw_pool.tile((P, CS), f32, name="xn", tag="xn")
                        nc.vector.tensor_scalar(
                            out=xn[:], in0=hraw[:, cs_sl],
                            scalar1=mv[:, 0:1], scalar2=rstd[:],
                            op0=ALU.subtract, op1=ALU.mult,
                        )
                        if affine[gname]:
                            nc.vector.tensor_tensor(
                                out=xn[:], in0=xn[:], in1=aff_s[gname][:, cs_sl],
                                op=ALU.mult,
                            )
                        if affine[bename]:
                            nc.vector.tensor_tensor(
                                out=xn[:], in0=xn[:], in1=aff_s[bename][:, cs_sl],
                                op=ALU.add,
                            )
                        nc.scalar.activation(xg[:, cs_sl], xn[:], AF.Gelu)
                return xg

            def transpose_to_hT(xg, nh, hT_tag):
                """PE-transpose (P, nh) bf16 -> (P, nh//P, P) feature-major."""
                nch = nh // CS
                hT = hT_pool.tile((P, nh // P, P), bf16, tag=hT_tag)
                for c in range(nch):
                    pt = tp_pool.tile((P, CS), bf16, name="pt", tag="pt")
                    for j in range(CS // P):
                        b = c * (CS // P) + j
                        nc.tensor.transpose(
                            pt[:, j * P:(j + 1) * P],
                            xg[:, b * P:(b + 1) * P],
                            identity[:],
                        )
                    nc.vector.tensor_copy(
                        hT[:, c * (CS // P):(c + 1) * (CS // P), :], pt[:]
                    )
                return hT

            xg1 = {}
            xg2 = {}

            def stage_a(i):
                w1_s = weights_for(i)[0]
                tok = slice(i * P, (i + 1) * P)
                xg1[i] = mm_ln_gelu(
                    i, lambda k: tT_s[:, k, tok], KD, w1_s, H,
                    "b1", "g1", "be1", "xg1",
                )

            def stage_b(i):
                w2_s = weights_for(i)[1]
                h1T = transpose_to_hT(xg1.pop(i), H, "hT1")
                xg2[i] = mm_ln_gelu(
                    i, lambda k: h1T[:, k, :], KH, w2_s, H,
                    "b2", "g2", "be2", "xg2",
                )

            def stage_c(i):
                w3_s = weights_for(i)[2]
                h2T = transpose_to_hT(xg2.pop(i), H, "hT2")
                ps3 = acc_pool.tile((P, D), f32, name="ps3", tag="ps_acc")
                for k in range(KH):
                    nc.tensor.matmul(
                        ps3[:], h2T[:, k, :], w3_s[:, k, :],
                        start=(k == 0), stop=(k == KH - 1),
                    )
                outt = out_pool.tile((P, D), f32, tag="outt")
                if affine["b3"]:
                    nc.vector.tensor_tensor(
                        out=outt[:], in0=ps3[:], in1=aff_s["b3"][:], op=ALU.add,
                    )
                    nc.scalar.mul(outt[:], outt[:], cw_s[:, i:i + 1])
                else:
                    nc.vector.tensor_scalar(
                        out=outt[:], in0=ps3[:],
                        scalar1=cw_s[:, i:i + 1], scalar2=None,
                        op0=ALU.mult, op1=ALU.bypass,
                    )
                tok = slice(i * P, (i + 1) * P)
                nc.gpsimd.dma_start(out_d[tok, :], outt[:])

            # prologue: queue several stage-A tiles so the PE has work
            # while the initial weight/token DMAs are still landing
            PD = min(1, n_tiles)
            for i in range(PD):
                stage_a(i)
            for j in range(n_tiles):
                if j + PD < n_tiles:
                    stage_a(j + PD)
                stage_b(j)
                # after the last segment-A consumer of W2 is issued,
                # stream segment-B W2 into the same SBUF tile
                if two_seg and j == n_a - 1:
                    for k in range(KH):
                        nc.sync.dma_start(w2a_s[:, k, :], w2b_d[k])
                if j >= 1:
                    stage_c(j - 1)
            stage_c(n_tiles - 1)

    nc.compile()
    return nc


def kernel(**inputs):
    global last_exec_time_ns
    import ml_dtypes

    from concourse import bass_utils

    inp = {k: np.asarray(v) for k, v in inputs.items()}
    x = inp["x"].astype(np.float32, copy=False)
    B, S, d = x.shape
    T = B * S
    t = x.reshape(T, d)

    top2, topv = _route(t, inp["Wg1"], inp["bg1"], inp["Wg2"], inp["bg2"])

    idx_per_e = []
    w_per_e = []
    for e in range(E):
        sel = np.nonzero(top2 == e)
        idx_per_e.append(sel[0])
        w_per_e.append(topv[sel].astype(np.float32))

    affine = {
        "b1": not np.all(inp["b1"] == 0.0),
        "g1": not np.all(inp["g1"] == 1.0),
        "be1": not np.all(inp["be1"] == 0.0),
        "b2": not np.all(inp["b2"] == 0.0),
        "g2": not np.all(inp["g2"] == 1.0),
        "be2": not np.all(inp["be2"] == 0.0),
        "b3": not np.all(inp["b3"] == 0.0),
    }
    any_affine = any(affine.values())

    tiles_e = [int(math.ceil(len(ix) / P)) for ix in idx_per_e]
    if any_affine:
        # affine params are per-expert; keep one expert per core
        sA, sB = max(max(tiles_e), 1), 0
        assign = [(1, 0)] * E
    else:
        sA, sB, assign = _pack_segments(tiles_e)

    # build slot lists: each slot = (expert, first_piece, n_pieces)
    slotsA, slotsB = [], []
    for e in range(E):
        a_e, b_e = assign[e]
        pos = 0
        nt = tiles_e[e]
        for _ in range(a_e):
            take = max(0, min(sA, nt - pos))
            slotsA.append((e, pos, take))
            pos += take
        for _ in range(b_e):
            take = max(0, min(sB, nt - pos))
            slotsB.append((e, pos, take))
            pos += take
    while len(slotsA) < E:
        slotsA.append((None, 0, 0))
    while len(slotsB) < E:
        slotsB.append((None, 0, 0))

    n_tiles = sA + sB
    C = n_tiles * P
    bf = ml_dtypes.bfloat16
    zW1 = np.zeros((D // P, P, H), bf)
    zW2 = np.zeros((H // P, P, H), bf)
    zW3 = np.zeros((H // P, P, D), bf)

    def slot_tokens(slot, s_cap):
        """token columns (D, s_cap*P) f32 + cw (s_cap*P,) for one slot."""
        e, pos, take = slot
        tt = np.zeros((D, s_cap * P), np.float32)
        cw = np.zeros((s_cap * P,), np.float32)
        if e is not None and take > 0:
            lo = pos * P
            hi = min(len(idx_per_e[e]), (pos + take) * P)
            n = hi - lo
            tt[:, :n] = t[idx_per_e[e][lo:hi]].T
            cw[:n] = w_per_e[e][lo:hi]
        return tt, cw

    def expert_w(e, which):
        if e is None:
            return (zW1, zW2, zW3)[which]
        w = (inp["W1"], inp["W2"], inp["W3"])[which][e]
        kk = (D // P, H // P, H // P)[which]
        return np.ascontiguousarray(w).reshape(kk, P, w.shape[1]).astype(bf)

    in_maps = []
    for c in range(E):
        ttA, cwA = slot_tokens(slotsA[c], sA)
        eA = slotsA[c][0]
        if sB > 0:
            ttB, cwB = slot_tokens(slotsB[c], sB)
            tt = np.concatenate([ttA, ttB], axis=1)
            cw = np.concatenate([cwA, cwB])
        else:
            tt, cw = ttA, cwA
        m = {
            "tT": tt.reshape(D // P, P, C).astype(bf),
            "W1a": expert_w(eA, 0),
            "W2a": expert_w(eA, 1),
            "W3a": expert_w(eA, 2),
            "cw": np.ascontiguousarray(cw.reshape(n_tiles, P).T).astype(np.float32),
        }
        if sB > 0:
            eB = slotsB[c][0]
            m["W1b"] = expert_w(eB, 0)
            m["W2b"] = expert_w(eB, 1)
            m["W3b"] = expert_w(eB, 2)
        for name in ("b1", "g1", "be1", "b2", "g2", "be2", "b3"):
            if affine[name]:
                row = np.asarray(inp[name][eA if eA is not None else 0], np.float32)
                m[name] = np.ascontiguousarray(np.broadcast_to(row, (P, row.shape[0])))
        in_maps.append(m)

    nc = _build_program(sA, sB, affine)

    trace = bool(os.environ.get("KERNEL_TRACE"))
    if trace:
        try:
            from antenv import axon_hooks as _ah  # noqa: F401
        except ImportError:
            trace = False
    try:
        res = bass_utils.run_bass_kernel_spmd(
            nc, in_maps, core_ids=list(range(E)), trace=trace
        )
    except Exception:
        if not trace:
            raise
        res = bass_utils.run_bass_kernel_spmd(
            nc, in_maps, core_ids=list(range(E)), trace=False
        )
    last_exec_time_ns = getattr(res, "exec_time_ns", None)

    out_full = np.zeros((T, D), np.float32)
    for c in range(E):
        o = np.asarray(res.results[c]["out"], np.float32)
        for si, (slot, s_cap, base) in enumerate(
            ((slotsA[c], sA, 0), (slotsB[c], sB, sA * P))
        ):
            e, pos, take = slot
            if e is None or take == 0:
                continue
            lo = pos * P
            hi = min(len(idx_per_e[e]), (pos + take) * P)
            n = hi - lo
            out_full[idx_per_e[e][lo:hi]] += o[base:base + n]
    return out_full.reshape(B, S, D).astype(np.float32)
